# revision 3
# baseline (speedup 1.0000x reference)
"""Trainium2 kernel for nn_DatasetScoreMatchingLoss.

Strategy (dataset axis sharded over 8 NeuronCores):
  * Host packs, per element i of the dataset buffers:
        combo c = group + 16*label          (24 live combos: 0..11, 16..27)
        v       = score + 2*c               (f32; windows [2c, 2c+1) carry the score)
    `c` ships as bf16 (exact for these small ints), `v` as f32.
  * Each core streams its 2,097,152-element shard and, per combo c0, computes
        counts: tensor_scalar  (c == c0)            with fused free-dim accum
        sums:   scalar_tensor_tensor (c == c0) * v  with fused free-dim accum
    accumulated per partition into an output matrix [128, 2*24*NT].
  * Host reduces partials in f64, recovers S_c = W_c - 2*c*N_c, applies the
    batch scatter as an O(B) correction (last-write-wins per index), and
    mirrors the reference's float32 variance arithmetic.

The batch scatter (32768 updates into 16.7M-element buffers) is applied as a
correction to the per-combo sums/counts instead of materializing the updated
buffers; duplicate indices resolve last-wins like XLA's sequential scatter.

Inputs that violate the fast-path assumptions (NaN scores, labels outside
{0,1}, groups outside [0,12)) fall back to an exact numpy implementation.
"""

import numpy as np

NUM_GROUPS = 16
MIN_COUNT = 10
N_CORES = 8
P = 128
FD = 8192
NT = 2
E_CORE = P * FD * NT          # 2,097,152
N_TOTAL = N_CORES * E_CORE    # 16,777,216
B = 32768
COMBOS = list(range(12)) + list(range(16, 28))
NCOMB = len(COMBOS)           # 24
NCOL = 2 * NCOMB * NT         # 96

_NC_CACHE = None


def _build_nc(fd=FD, nt=NT):
    import concourse.bass as bass
    import concourse.mybir as mybir

    e_core = P * fd * nt
    ncol = 2 * NCOMB * nt
    nc = bass.Bass()
    v_in = nc.declare_dram_parameter("v", [e_core], mybir.dt.float32, isOutput=False)
    c_in = nc.declare_dram_parameter("c", [e_core], mybir.dt.bfloat16, isOutput=False)
    acc_out = nc.declare_dram_parameter("acc", [P, ncol], mybir.dt.float32, isOutput=True)
    v_t = v_in.rearrange("(t p f) -> t p f", t=nt, p=P)
    c_t = c_in.rearrange("(t p f) -> t p f", t=nt, p=P)

    v_sb = [nc.alloc_sbuf_tensor(f"v_sb{t}", [P, fd], mybir.dt.float32).ap() for t in range(nt)]
    c_sb = [nc.alloc_sbuf_tensor(f"c_sb{t}", [P, fd], mybir.dt.bfloat16).ap() for t in range(nt)]
    scr_f = nc.alloc_sbuf_tensor("scr_f", [P, fd], mybir.dt.float32).ap()
    scr_b = nc.alloc_sbuf_tensor("scr_b", [P, fd], mybir.dt.bfloat16).ap()
    acc = nc.alloc_sbuf_tensor("acc_sb", [P, ncol], mybir.dt.float32).ap()
    NT_, FD_ = nt, fd

    with (
        nc.Block() as block,
        nc.semaphore("tile_sem") as tile_sem,
        nc.semaphore("done_sem") as done_sem,
        nc.semaphore("out_sem") as out_sem,
    ):
        @block.sync
        def _(sync: bass.BassEngine):
            for t in range(NT_):
                sync.dma_start(out=v_sb[t][:], in_=v_t[t]).then_inc(tile_sem, 16)
                sync.dma_start(out=c_sb[t][:], in_=c_t[t]).then_inc(tile_sem, 16)
            sync.wait_ge(done_sem, 1)
            sync.dma_start(out=acc_out[:], in_=acc[:]).then_inc(out_sem, 16)
            sync.wait_ge(out_sem, 16)

        @block.vector
        def _(vector: bass.BassEngine):
            col = 0
            for t in range(NT_):
                vector.wait_ge(tile_sem, 32 * (t + 1))
                for c0 in COMBOS:
                    vector.tensor_scalar(
                        out=scr_b[:], in0=c_sb[t][:],
                        scalar1=float(c0), scalar2=None,
                        op0=mybir.AluOpType.is_equal, op1=mybir.AluOpType.add,
                        accum_out=acc[:, col:col + 1])
                    col += 1
                    vector.scalar_tensor_tensor(
                        out=scr_f[:], in0=c_sb[t][:], scalar=float(c0), in1=v_sb[t][:],
                        op0=mybir.AluOpType.is_equal, op1=mybir.AluOpType.mult,
                        accum_out=acc[:, col:col + 1])
                    col += 1
            vector.engine_nop().then_inc(done_sem, 1)

    return nc


def _get_nc():
    global _NC_CACHE
    if _NC_CACHE is None:
        _NC_CACHE = _build_nc()
    return _NC_CACHE


def _final_loss_f32(S, N):
    """Mirror the reference's float32 arithmetic. S, N are f64 arrays of 32
    combo bins (c = g + 16*l)."""
    f = np.float32
    losses = {}
    ns = {}
    for lab in (0, 1):
        sums = np.zeros(NUM_GROUPS, np.float32)
        cnts = np.zeros(NUM_GROUPS, np.float32)
        sums[:NUM_GROUPS] = S[16 * lab:16 * lab + NUM_GROUPS].astype(np.float32)
        cnts[:NUM_GROUPS] = N[16 * lab:16 * lab + NUM_GROUPS].astype(np.float32)
        avg = sums / np.maximum(cnts, f(1.0))
        incl = (cnts >= f(MIN_COUNT)).astype(np.float32)
        n = incl.sum(dtype=np.float32)
        mean = (avg * incl).sum(dtype=np.float32) / np.maximum(n, f(1.0))
        d = avg - mean
        var = (incl * d * d).sum(dtype=np.float32) / np.maximum(n - f(1.0), f(1.0))
        losses[lab] = var
        ns[lab] = n
    pos_ok = ns[1] >= 2.0
    neg_ok = ns[0] >= 2.0
    if pos_ok and neg_ok:
        loss = f(0.5) * (losses[1] + losses[0])
    elif pos_ok:
        loss = losses[1]
    elif neg_ok:
        loss = losses[0]
    else:
        loss = f(0.0)
    return np.float32(loss)


def _numpy_fallback(probs, labels, groups, indices, score_buffer, label_buffer, group_buffer):
    scores = score_buffer.copy()
    labs = label_buffer.copy()
    grps = group_buffer.copy()
    scores[indices] = probs
    labs[indices] = labels
    grps[indices] = groups
    valid = (~np.isnan(scores)) & (labs >= 0) & (grps >= 0)
    scores = np.where(valid, scores, 0.0).astype(np.float32)
    seg = np.where(valid, grps, 0)
    S = np.zeros(32, np.float64)
    N = np.zeros(32, np.float64)
    for lab in (0, 1):
        w = (valid & (labs == lab))
        S[16 * lab:16 * lab + 16] = np.bincount(
            seg, weights=np.where(w, scores, 0.0).astype(np.float64), minlength=16)[:16]
        N[16 * lab:16 * lab + 16] = np.bincount(
            seg, weights=w.astype(np.float64), minlength=16)[:16]
    return _final_loss_f32(S, N)


def _batch_correction(probs, labels, groups, indices, score_buffer, label_buffer, group_buffer):
    """Per-combo (c = g + 16*l) delta sums/counts from applying the batch
    scatter, last-write-wins per index. Returns (dS[32], dN[32]) in f64."""
    rev = indices[::-1]
    upos, first_in_rev = np.unique(rev, return_index=True)
    winner = (B - 1) - first_in_rev          # original batch index whose write wins

    dS = np.zeros(33, np.float64)
    dN = np.zeros(33, np.float64)

    # remove old contributions at touched positions
    so = score_buffer[upos].astype(np.float64)
    lo = label_buffer[upos]
    go = group_buffer[upos]
    old_ok = (~np.isnan(so)) & (lo >= 0) & (lo <= 1) & (go >= 0) & (go < 16)
    c_old = np.where(old_ok, go + 16 * lo, 32)
    np.subtract.at(dN, c_old, 1.0)
    np.subtract.at(dS, c_old, np.where(old_ok, so, 0.0))

    # add new contributions
    sn = probs[winner].astype(np.float64)
    ln = labels[winner]
    gn = groups[winner]
    new_ok = (~np.isnan(sn)) & (ln >= 0) & (ln <= 1) & (gn >= 0) & (gn < 16)
    c_new = np.where(new_ok, gn + 16 * ln, 32)
    np.add.at(dN, c_new, 1.0)
    np.add.at(dS, c_new, np.where(new_ok, sn, 0.0))

    return dS[:32], dN[:32]


def kernel(probs, labels, groups, indices, score_buffer, label_buffer, group_buffer):
    probs = np.asarray(probs)
    labels = np.asarray(labels)
    groups = np.asarray(groups)
    indices = np.asarray(indices)
    score_buffer = np.asarray(score_buffer)
    label_buffer = np.asarray(label_buffer)
    group_buffer = np.asarray(group_buffer)

    clean = (
        score_buffer.size == N_TOTAL
        and int(label_buffer.min()) >= 0 and int(label_buffer.max()) <= 1
        and int(group_buffer.min()) >= 0 and int(group_buffer.max()) < 12
        and not np.isnan(score_buffer).any()
    )
    if not clean:
        return _numpy_fallback(probs, labels, groups, indices,
                               score_buffer, label_buffer, group_buffer)

    from concourse.bass_utils import run_bass_kernel_spmd
    import ml_dtypes

    # host packing: c = g + 16*l (bf16-exact small ints), v = s + 2*c
    c_i = group_buffer + (label_buffer << 4)
    c_f = c_i.astype(np.float32)
    v = score_buffer + 2.0 * c_f
    c_bf = c_f.view(np.uint32)
    c_bf = (c_bf >> np.uint32(16)).astype(np.uint16).view(ml_dtypes.bfloat16)

    in_maps = []
    for r in range(N_CORES):
        sl = slice(r * E_CORE, (r + 1) * E_CORE)
        in_maps.append({"v": v[sl], "c": c_bf[sl]})

    nc = _get_nc()
    res = run_bass_kernel_spmd(nc, in_maps, list(range(N_CORES)))

    # combine per-partition partials (f64)
    W = np.zeros(32, np.float64)   # sum of v*[c==c0]
    N = np.zeros(32, np.float64)   # counts
    for r in range(N_CORES):
        acc = res.results[r]["acc"].astype(np.float64)   # [128, NCOL]
        col = 0
        for t in range(NT):
            for c0 in COMBOS:
                N[c0] += acc[:, col].sum()
                W[c0] += acc[:, col + 1].sum()
                col += 2
    S = W - 2.0 * np.arange(32, dtype=np.float64) * N

    dS, dN = _batch_correction(probs, labels, groups, indices,
                               score_buffer, label_buffer, group_buffer)
    S += dS
    N += dN

    return _final_loss_f32(S, N)
